# revision 34
# baseline (speedup 1.0000x reference)
"""Causal self-attention (B=4, T=2048, C=1024, H=16) on 8 Trainium2 NeuronCores.

Sharding: tensor-parallel over heads. Each core owns 2 heads:
  - Wq/Wk/Wv column slices [C, 128], Wo row slice [128, C]
  - computes q/k/v for its heads from the full x, flash-style causal
    attention, and a partial output projection (fp16).
  - host sums the 8 partial outputs in fp32 and adds bo.

Device algorithm per batch b:
  xT[b] (host-pretransposed [C, T] fp16) DMA'd in 128-row tiles.
  qT/kT/vT [128(2h x 64d), T] = W.T @ xT via paired 2-bank PSUM tiles
  (+bias on DVE). va tiles [128 t, 65] = [64 v-dims | ones] per head via
  SBUF->SBUF DMA transpose (XBAR) of vT 128-blocks; ones col memset.
  Scores (transposed): per i-chunk of 512, per j-tile of 128:
  ps_pair [128, 1024] (h0 cols 0:512, h1 512:1024) = kT.T @ qT, with
  diagonal-block matmuls narrowed to the causal width. One ACT exp per
  pair -> e fp16 SBUF; upper-tri mask multiply (DVE) on the diagonal
  128-col block. attV: py_h [65, 512] PSUM += va_h.T @ e_h, row 64 is
  the softmax denominator (ones column).
  Normalize: r = exp(-ln(denom)) on ACT (activation table pinned to the
  set containing both Ln and Exp, so no table reloads), DMA p64->p0 +
  gpsimd partition_broadcast -> ab [64, 1024]; yta = py * ab (DVE), h1
  DMA-moved to partitions 64:128 of yta_pair for K=128 out-proj.
  Out-proj (deferred one i-chunk, popped after the next epilogue): po
  pairs rotate through the score PSUM ring; fp32 DVE copies; the gpsimd
  SWDGE out-DMA casts fp32->fp16 in flight. Careful DMA-queue placement
  throughout (see inline comments): moving small moves or transposes to
  other queues exposes completion-count semaphore races.
"""

import sys

if "/opt/trn_rl_repo" not in sys.path:
    sys.path.insert(0, "/opt/trn_rl_repo")

from contextlib import ExitStack

import numpy as np

import concourse.bass as bass
import concourse.tile as tile
from concourse import bacc, mybir
from concourse import bass_utils

B, T, C, H, D = 4, 2048, 1024, 16, 64
N_CORES = 8
HPC = H // N_CORES  # heads per core = 2
W = HPC * D  # per-core projection width = 128

F32 = mybir.dt.float32
F16 = mybir.dt.float16
AF = mybir.ActivationFunctionType

ICH = 512  # i (query) chunk in the free dim
NIC = T // ICH  # 4
NKT = C // 128  # 8 contraction tiles for projections
NJT = T // 128  # 16 key tiles

_CACHE = {}


def _kernel_body(ctx, tc, xT, wq, wk, wv, wo, bq, bk, bv, trimask, out):
    nc = tc.nc

    const_p = ctx.enter_context(tc.tile_pool(name="const", bufs=1))
    w_p = ctx.enter_context(tc.tile_pool(name="wts", bufs=1))
    xt_p = ctx.enter_context(tc.tile_pool(name="xt", bufs=2 * NKT))
    act_p = ctx.enter_context(tc.tile_pool(name="acts", bufs=4))
    va_p = ctx.enter_context(tc.tile_pool(name="vaug", bufs=72))
    e_p = ctx.enter_context(tc.tile_pool(name="ep", bufs=4))
    yta_p = ctx.enter_context(tc.tile_pool(name="yta", bufs=3))
    y1_p = ctx.enter_context(tc.tile_pool(name="y1", bufs=2))
    r_p = ctx.enter_context(tc.tile_pool(name="rp", bufs=3))
    ab_p = ctx.enter_context(tc.tile_pool(name="ab", bufs=2))
    ob_p = ctx.enter_context(tc.tile_pool(name="ob", bufs=3))
    # PSUM: sc ring (scores/v-proj/out-proj pairs) 2x2 banks + py pairs 2x2 = 8
    sc_p = ctx.enter_context(tc.tile_pool(name="sc", bufs=2, space="PSUM"))
    py_p = ctx.enter_context(tc.tile_pool(name="py", bufs=2, space="PSUM"))

    # pin the activation table to the set containing both Ln and Exp, so the
    # softmax exps and the Ln/Exp reciprocal never reload tables
    from concourse.hw_specs import get_activation_tables

    tables = list(get_activation_tables(nc.m.arch).items())
    set_id = next(
        i for i, (_, funcs) in enumerate(tables) if AF.Exp in funcs and AF.Ln in funcs
    )
    nc.scalar.add_instruction(
        mybir.InstLoadActFuncSet(
            name=nc.get_next_instruction_name(), ins=[], outs=[], act_func_set_id=set_id
        )
    )

    # constants / weights (loaded once)
    bias_q = const_p.tile([W, 1], F32, tag="bq")
    bias_k = const_p.tile([W, 1], F32, tag="bk")
    bias_v = const_p.tile([W, 1], F32, tag="bv")
    tri = const_p.tile([128, 128], F16, tag="tri")
    nc.sync.dma_start(bias_q[:], bq[:])
    nc.sync.dma_start(bias_k[:], bk[:])
    nc.sync.dma_start(bias_v[:], bv[:])
    nc.sync.dma_start(tri[:], trimask[:])

    wq_sb = w_p.tile([128, C], F16, tag="wq")
    wk_sb = w_p.tile([128, C], F16, tag="wk")
    wv_sb = w_p.tile([128, C], F16, tag="wv")
    for kt in range(NKT):
        sl = slice(kt * 128, (kt + 1) * 128)
        nc.sync.dma_start(wq_sb[:, sl], wq[sl, :])
        nc.scalar.dma_start(wk_sb[:, sl], wk[sl, :])
        nc.gpsimd.dma_start(wv_sb[:, sl], wv[sl, :])
    wo_sb = w_p.tile([128, C], F16, tag="wo")
    nc.scalar.dma_start(wo_sb[:], wo[:])

    pending = []  # deferred out-projection closures (one per i-tile)

    def pop_pending():
        if pending:
            pending.pop(0)()

    # prefetch batch 0 x tiles
    xts = {}
    for kt in range(NKT):
        xt = xt_p.tile([128, T], F16, tag="xt")
        nc.sync.dma_start(xt[:], xT[0, kt * 128 : (kt + 1) * 128, :])
        xts[(0, kt)] = xt

    for b in range(B):
        # ---- QKV projections ----
        qT = act_p.tile([128, T], F16, tag="qT")
        kT = act_p.tile([128, T], F16, tag="kT")
        vT = act_p.tile([128, T], F16, tag="vT")
        vas = []  # [(va_h0, va_h1)] per j-tile
        for n in range(NIC):
            csl = slice(n * ICH, (n + 1) * ICH)
            qk_ps = sc_p.tile([128, 2 * ICH], F32, tag="sc")
            v_ps = sc_p.tile([128, 2 * ICH], F32, tag="sc")
            for kt in range(NKT):
                wsl = slice(kt * 128, (kt + 1) * 128)
                st, sp = kt == 0, kt == NKT - 1
                x_k = xts[(b, kt)]
                nc.tensor.matmul(qk_ps[:, 0:ICH], wq_sb[:, wsl], x_k[:, csl], start=st, stop=sp)
                nc.tensor.matmul(qk_ps[:, ICH : 2 * ICH], wk_sb[:, wsl], x_k[:, csl], start=st, stop=sp)
                nc.tensor.matmul(v_ps[:, 0:ICH], wv_sb[:, wsl], x_k[:, csl], start=st, stop=sp)
            if n == 1:
                pop_pending()
            if n == 2:
                pop_pending()
                pop_pending()
            if n == 3:
                pop_pending()
            # vT bias on DVE: the va transposes (issued from the sync/scalar
            # DGE queues) get proper cross-engine semaphores on a DVE
            # producer; a same-engine ACT producer races the scalar DGE.
            # q/k biases go to ACT (idle during QKV, Identity in the pinned
            # table, nothing DMA-reads them).
            nc.vector.tensor_scalar_add(vT[:, csl], v_ps[:, 0:ICH], bias_v[:])
            nc.scalar.activation(qT[:, csl], qk_ps[:, 0:ICH], AF.Identity, bias=bias_q[:])
            nc.scalar.activation(kT[:, csl], qk_ps[:, ICH : 2 * ICH], AF.Identity, bias=bias_k[:])
            # va tiles for this chunk's 4 j-tiles (DMA transpose + ones col),
            # split across both HWDGE queues
            for tt in range(n * 4, n * 4 + 4):
                tsl = slice(tt * 128, (tt + 1) * 128)
                va0 = va_p.tile([128, 65], F16, tag="va")
                va1 = va_p.tile([128, 65], F16, tag="va")
                nc.sync.dma_start_transpose(va0[:, 0:64], vT[0:64, tsl])
                nc.scalar.dma_start_transpose(va1[:, 0:64], vT[64:128, tsl])
                nc.gpsimd.memset(va0[:, 64:65], 1.0)
                nc.gpsimd.memset(va1[:, 64:65], 1.0)
                vas.append((va0, va1))

        # prefetch next batch's x while attention runs (gpsimd queue: cheap issue)
        if b + 1 < B:
            for kt in range(NKT):
                xt = xt_p.tile([128, T], F16, tag="xt")
                nc.gpsimd.dma_start(xt[:], xT[b + 1, kt * 128 : (kt + 1) * 128, :])
                xts[(b + 1, kt)] = xt

        # ---- attention per i-chunk ----
        for ic in range(NIC):
            i0 = ic * ICH
            njt = (i0 + ICH) // 128

            def jt_width(jt, i0=i0):
                j0 = jt * 128
                if j0 <= i0 - 128:
                    return ICH  # fully below diagonal
                return i0 + ICH - j0  # diagonal block, narrowed

            def emit_scores(jt, i0=i0):
                wdt = jt_width(jt)
                j0 = jt * 128
                jsl = slice(j0, j0 + 128)
                isl = slice(i0 + ICH - wdt, i0 + ICH)
                ps = sc_p.tile([128, 2 * ICH], F32, tag="sc")
                nc.tensor.matmul(
                    ps[:, 0:wdt], kT[0:64, jsl], qT[0:64, isl], start=True, stop=True
                )
                nc.tensor.matmul(
                    ps[:, ICH : ICH + wdt], kT[64:128, jsl], qT[64:128, isl],
                    start=True, stop=True,
                )
                return ps

            py = py_p.tile([65, 2 * ICH], F32, tag="py")

            ps_q = [emit_scores(0)]
            if njt > 1:
                ps_q.append(emit_scores(1))
            for jt in range(njt):
                wdt = jt_width(jt)
                ps = ps_q.pop(0)
                e = e_p.tile([128, 2 * ICH], F16, tag="e")
                if wdt == ICH:
                    nc.scalar.activation(e[:, 0 : 2 * ICH], ps[:, 0 : 2 * ICH], AF.Exp)
                else:
                    e_v = e[:, :].rearrange("p (g w) -> p g w", g=2)[:, :, 0:wdt]
                    ps_v = ps[:, :].rearrange("p (g w) -> p g w", g=2)[:, :, 0:wdt]
                    nc.scalar.activation(e_v, ps_v, AF.Exp)
                diag = jt * 128 >= i0
                if diag:
                    nc.vector.tensor_mul(e[:, 0:128], e[:, 0:128], tri[:])
                    nc.vector.tensor_mul(e[:, ICH : ICH + 128], e[:, ICH : ICH + 128], tri[:])
                if jt + 2 < njt:
                    ps_q.append(emit_scores(jt + 2))
                st, sp = jt == 0, jt == njt - 1
                off = ICH - wdt
                nc.tensor.matmul(
                    py[:, off:ICH], vas[jt][0][:, 0:65], e[:, 0:wdt], start=st, stop=sp
                )
                nc.tensor.matmul(
                    py[:, ICH + off : 2 * ICH], vas[jt][1][:, 0:65],
                    e[:, ICH : ICH + wdt], start=st, stop=sp,
                )

            # ---- normalize: r = exp(-ln(denom)) on ACT (shared table), then
            # DMA p64->p0, gpsimd partition broadcast, DVE scale ----
            lnd = r_p.tile([65, 2 * ICH], F32, tag="lnd")
            rr = r_p.tile([65, 2 * ICH], F32, tag="rr")
            nc.scalar.activation(lnd[64:65, :], py[64:65, :], AF.Ln)
            nc.scalar.activation(rr[64:65, :], lnd[64:65, :], AF.Exp, scale=-1.0)
            rp = r_p.tile([1, 2 * ICH], F32, tag="rp0")
            nc.gpsimd.dma_start(rp[:], rr[64:65, :])
            ab = ab_p.tile([64, 2 * ICH], F32, tag="ab")
            nc.gpsimd.partition_broadcast(ab[:], rp[:])
            yta = yta_p.tile([128, ICH], F16, tag="yta")
            y1t = y1_p.tile([64, ICH], F16, tag="y1t")
            nc.vector.tensor_mul(yta[0:64, :], py[0:64, 0:ICH], ab[:, 0:ICH])
            nc.vector.tensor_mul(y1t[:], py[0:64, ICH : 2 * ICH], ab[:, ICH : 2 * ICH])
            nc.gpsimd.dma_start(yta[64:128, :], y1t[:])
            # out-projection of the previous i-chunk, after the epilogue so
            # its out-DMAs queue behind this chunk's y1t move
            pop_pending()

            # ---- deferred out-projection for this i-chunk ----
            # po pairs rotate through the score ring; fp32 copies, the
            # gpsimd SWDGE out-DMA casts fp32 -> fp16 in flight.
            def _outproj(b=b, ic=ic, yta=yta):
                for itl in range(ICH // 128):
                    tsl = slice(itl * 128, (itl + 1) * 128)
                    it = ic * 4 + itl
                    ob = ob_p.tile([128, C], F32, tag="ob")
                    po = sc_p.tile([128, 2 * ICH], F32, tag="sc")
                    nc.tensor.matmul(
                        po[:, 0:ICH], yta[:, tsl], wo_sb[:, 0:ICH], start=True, stop=True
                    )
                    nc.tensor.matmul(
                        po[:, ICH : 2 * ICH], yta[:, tsl], wo_sb[:, ICH : 2 * ICH],
                        start=True, stop=True,
                    )
                    nc.vector.tensor_copy(ob[:, 0:ICH], po[:, 0:ICH])
                    nc.vector.tensor_copy(ob[:, ICH : 2 * ICH], po[:, ICH : 2 * ICH])
                    nc.gpsimd.dma_start(out[b, it * 128 : (it + 1) * 128, :], ob[:])

            pending.append(_outproj)

    while pending:
        pending.pop(0)()


def _build():
    if "nc" in _CACHE:
        return _CACHE["nc"]
    nc = bacc.Bacc("TRN2", target_bir_lowering=False, debug=False, num_devices=N_CORES)
    xT = nc.dram_tensor("xT", [B, C, T], F16, kind="ExternalInput").ap()
    wq = nc.dram_tensor("wq", [C, W], F16, kind="ExternalInput").ap()
    wk = nc.dram_tensor("wk", [C, W], F16, kind="ExternalInput").ap()
    wv = nc.dram_tensor("wv", [C, W], F16, kind="ExternalInput").ap()
    wo = nc.dram_tensor("wo", [W, C], F16, kind="ExternalInput").ap()
    bq = nc.dram_tensor("bq", [W, 1], F32, kind="ExternalInput").ap()
    bk = nc.dram_tensor("bk", [W, 1], F32, kind="ExternalInput").ap()
    bv = nc.dram_tensor("bv", [W, 1], F32, kind="ExternalInput").ap()
    trimask = nc.dram_tensor("trimask", [128, 128], F16, kind="ExternalInput").ap()
    out = nc.dram_tensor("out", [B, T, C], F16, kind="ExternalOutput").ap()

    with tile.TileContext(nc) as tc:
        with ExitStack() as ctx:
            _kernel_body(ctx, tc, xT, wq, wk, wv, wo, bq, bk, bv, trimask, out)
    nc.compile()
    _CACHE["nc"] = nc
    return nc


def make_in_maps(inputs):
    x = np.asarray(inputs["x"], np.float32)
    Wq = np.asarray(inputs["Wq"], np.float32)
    bq = np.asarray(inputs["bq"], np.float32)
    Wk = np.asarray(inputs["Wk"], np.float32)
    bk = np.asarray(inputs["bk"], np.float32)
    Wv = np.asarray(inputs["Wv"], np.float32)
    bv = np.asarray(inputs["bv"], np.float32)
    Wo = np.asarray(inputs["Wo"], np.float32)

    scale = np.float32(1.0 / np.sqrt(D))
    xT = np.ascontiguousarray(x.transpose(0, 2, 1)).astype(np.float16)  # [B, C, T]
    Wq_s = Wq * scale
    bq_s = bq * scale
    trimask = np.triu(np.ones((128, 128), np.float16))

    in_maps = []
    for c in range(N_CORES):
        s = slice(c * W, (c + 1) * W)
        in_maps.append(
            {
                "xT": xT,
                "wq": np.ascontiguousarray(Wq_s[:, s]).astype(np.float16),
                "wk": np.ascontiguousarray(Wk[:, s]).astype(np.float16),
                "wv": np.ascontiguousarray(Wv[:, s]).astype(np.float16),
                "wo": np.ascontiguousarray(Wo[s, :]).astype(np.float16),
                "bq": np.ascontiguousarray(bq_s[s, None]),
                "bk": np.ascontiguousarray(bk[s, None]),
                "bv": np.ascontiguousarray(bv[s, None]),
                "trimask": trimask,
            }
        )
    return in_maps


def kernel(**inputs):
    nc = _build()
    in_maps = make_in_maps(inputs)
    res = bass_utils.run_bass_kernel_spmd(nc, in_maps, core_ids=list(range(N_CORES)))
    bo = np.asarray(inputs["bo"], np.float32)
    out = np.zeros((B, T, C), np.float32)
    for c in range(N_CORES):
        out += res.results[c]["out"].astype(np.float32)
    out += bo
    return out


if __name__ == "__main__":
    rng = np.random.default_rng(0)
    ins = {
        "x": rng.standard_normal((B, T, C), dtype=np.float32),
        "Wq": rng.standard_normal((C, C), dtype=np.float32) / 32,
        "bq": rng.standard_normal((C,), dtype=np.float32) * 0.02,
        "Wk": rng.standard_normal((C, C), dtype=np.float32) / 32,
        "bk": rng.standard_normal((C,), dtype=np.float32) * 0.02,
        "Wv": rng.standard_normal((C, C), dtype=np.float32) / 32,
        "bv": rng.standard_normal((C,), dtype=np.float32) * 0.02,
        "Wo": rng.standard_normal((C, C), dtype=np.float32) / 32,
        "bo": rng.standard_normal((C,), dtype=np.float32) * 0.02,
    }
    got = kernel(**ins)
    print("kernel ran, out shape", got.shape)
